# revision 10
# baseline (speedup 1.0000x reference)
"""APPNP (2-layer MLP + 10-step propagation) on 8 TRN2 NeuronCores.

Sharding: destination nodes row-partitioned across 8 cores (12500 each);
W1/b1/W2/b2 replicated. Per step, h~ = dinv*x_k is all-gathered (350KB/core),
each core gathers its in-edges' source features via GPSIMD ap_gather from a
chunked SBUF table, segment-sums them with a DVE prefix scan + run-boundary
gather + diff, combines chunk partials on the TensorEngine, and applies the
APPNP update. The MLP runs on the TensorEngine in bf16 from host-transposed x.

Self-contained: hardcodes N=100000, E=3200000, K=10, alpha=0.1, 8 cores.
"""
import os
import numpy as np

N_NODES = 100000
K_STEPS = int(os.environ.get("APPNP_KSTEPS", "10"))
CUT = int(os.environ.get("APPNP_CUT", "3"))
ALPHA = 0.1
N_CORES = 8
D_IN = 1433
D_HID = 64
D_OUT = 7

D_IN_PAD = 1536                            # 12 x 128
ROWS = N_NODES // N_CORES                  # 12500 dsts per core
CHUNK = 25000                              # src nodes per gather chunk
HALF = ROWS // 2                           # 6250 dsts per group
DST_Q = 192                                # dst runs per gather tile (%16)
N_TILES = (HALF + DST_Q - 1) // DST_Q      # 22
BCOLS = DST_Q + 1
CCH = 512                                  # postscale column chunk


def _wrap16(a2d):
    """[16k cols] per-group idx list -> ap_gather wrap layout rows."""
    blk = a2d.shape[-1] // 16
    return a2d.reshape(blk, 16).T


def _build_core_plan(src, dst_local, T_e):
    g_chunk = src // CHUNK
    g_half = dst_local // HALF
    grp = g_chunk + 4 * g_half
    dig = dst_local % HALF
    tile = dig // DST_Q
    pos = (src % CHUNK) + 1                # +1: zero slot at chunk pos 0

    eidx = np.zeros((128, (T_e // 16) * N_TILES), np.int16)
    bidx = np.zeros((128, (DST_Q // 16) * N_TILES), np.int16)
    for g in range(8):
        mg = grp == g
        dig_g, tile_g, pos_g = dig[mg], tile[mg], pos[mg]
        for t in range(N_TILES):
            mt = tile_g == t
            dig_t = dig_g[mt]
            order = np.argsort(dig_t, kind="stable")
            p = pos_g[mt][order].astype(np.int16)
            n = p.size
            assert n <= T_e, (n, T_e)
            col = np.zeros(T_e, np.int16)
            col[:n] = p
            ec = T_e // 16
            eidx[16 * g:16 * g + 16, t * ec:(t + 1) * ec] = _wrap16(col)
            dsort = dig_t[order] - t * DST_Q
            counts = np.bincount(dsort, minlength=DST_Q)
            ends = np.cumsum(counts).astype(np.int16)   # scan col of run end (1-based)
            bc = DST_Q // 16
            bidx[16 * g:16 * g + 16, t * bc:(t + 1) * bc] = _wrap16(ends)
    return eidx, bidx


def _build_bass(T_e):
    import concourse.bacc as bacc
    import concourse.mybir as mybir
    import concourse.tile as tile
    from contextlib import ExitStack

    f32 = mybir.dt.float32
    bf16 = mybir.dt.bfloat16
    i16 = mybir.dt.int16
    Alu = mybir.AluOpType
    Act = mybir.ActivationFunctionType

    nc = bacc.Bacc("TRN2", target_bir_lowering=False, debug=False,
                   num_devices=N_CORES)

    xT_ext = nc.dram_tensor("xT", [D_IN_PAD, ROWS], bf16, kind="ExternalInput")
    w1_ext = nc.dram_tensor("w1", [128, (D_IN_PAD // 128) * D_HID], bf16, kind="ExternalInput")
    b1_ext = nc.dram_tensor("b1", [D_HID, 1], f32, kind="ExternalInput")
    w2_ext = nc.dram_tensor("w2", [D_HID, D_OUT], bf16, kind="ExternalInput")
    b2_ext = nc.dram_tensor("b2", [D_OUT, 1], f32, kind="ExternalInput")
    dinv_ext = nc.dram_tensor("dinv7", [D_OUT, ROWS], f32, kind="ExternalInput")
    eidx_ext = nc.dram_tensor("eidx", [128, (T_e // 16) * N_TILES], i16, kind="ExternalInput")
    bidx_ext = nc.dram_tensor("bidx", [128, (DST_Q // 16) * N_TILES], i16, kind="ExternalInput")
    sel_ext = nc.dram_tensor("sel", [128, 2 * D_OUT], f32, kind="ExternalInput")
    out_ext = nc.dram_tensor("out", [D_OUT, ROWS], f32, kind="ExternalOutput")

    KCH = D_IN_PAD // 128                   # 12
    NRB = (ROWS + CCH - 1) // CCH           # 25 row blocks of 512

    with tile.TileContext(nc) as tc, ExitStack() as ctx:
        sb = ctx.enter_context(tc.tile_pool(name="sb", bufs=1))
        sb2 = ctx.enter_context(tc.tile_pool(name="sb2", bufs=2))
        ps = ctx.enter_context(tc.tile_pool(name="ps", bufs=2, space="PSUM"))
        ps1 = ctx.enter_context(tc.tile_pool(name="ps1", bufs=1, space="PSUM"))
        dram = ctx.enter_context(tc.tile_pool(name="dram", bufs=1, space="DRAM"))

        table = sb.tile([128, CHUNK + 1], f32, name="table")
        bdiff = sb.tile([128, DST_Q * N_TILES], f32, name="bdiff")
        eidx = sb.tile([128, (T_e // 16) * N_TILES], i16, name="eidx")
        bidx = sb.tile([128, (DST_Q // 16) * N_TILES], i16, name="bidx")
        selw = sb.tile([128, 2 * D_OUT], f32, name="selw")
        w1s = sb.tile([128, KCH * D_HID], bf16, name="w1s")
        w2s = sb.tile([D_HID, D_OUT], bf16, name="w2s")
        b1s = sb.tile([D_HID, 1], f32, name="b1s")
        b2s = sb.tile([D_OUT, 1], f32, name="b2s")
        ones17 = sb.tile([1, D_OUT], f32, name="ones17")
        ones71 = sb.tile([D_OUT, 1], f32, name="ones71")

        ht_d = dram.tile([D_OUT, ROWS], f32, name="ht_d")
        ag_ds = [dram.tile([N_CORES * D_OUT, ROWS], f32, name=f"ag_d{s}",
                           addr_space="Shared") for s in range(K_STEPS)]
        ah_d = dram.tile([D_OUT, ROWS], f32, name="ah_d")
        x_d = dram.tile([D_OUT, ROWS], f32, name="x_d")

        nc.sync.dma_start(eidx[:], eidx_ext.ap())
        nc.sync.dma_start(bidx[:], bidx_ext.ap())
        nc.sync.dma_start(selw[:], sel_ext.ap())
        nc.sync.dma_start(w1s[:], w1_ext.ap())
        nc.sync.dma_start(w2s[:], w2_ext.ap())
        nc.sync.dma_start(b1s[:], b1_ext.ap())
        nc.sync.dma_start(b2s[:], b2_ext.ap())
        nc.gpsimd.memset(table[:], 0.0)
        nc.gpsimd.memset(ones17[:], 1.0)
        nc.gpsimd.memset(ones71[:], 1.0)

        # ---------------- phase A: h2 = relu(x@W1+b1)@W2+b2 ----------------
        for rb in range(NRB):
            c0 = rb * CCH
            w = min(CCH, ROWS - c0)
            h1p = ps.tile([D_HID, CCH], f32, tag="h1p")
            for kc in range(KCH):
                xt = sb2.tile([128, CCH], bf16, tag="xrhs")
                nc.sync.dma_start(xt[:, :w], xT_ext.ap()[kc * 128:(kc + 1) * 128,
                                                         c0:c0 + w])
                nc.tensor.matmul(h1p[:, :w], w1s[:, kc * D_HID:(kc + 1) * D_HID],
                                 xt[:, :w], start=(kc == 0), stop=(kc == KCH - 1))
            h1s = sb2.tile([D_HID, CCH], bf16, tag="h1s")
            nc.scalar.activation(h1s[:, :w], h1p[:, :w], Act.Relu, bias=b1s[:])
            h2p = ps.tile([D_OUT, CCH], f32, tag="h2p")
            nc.tensor.matmul(h2p[:, :w], w2s[:], h1s[:, :w], start=True, stop=True)
            h2s = sb2.tile([D_OUT, CCH], f32, tag="h2s")
            nc.vector.tensor_scalar_add(h2s[:, :w], h2p[:, :w], b2s[:])
            if CUT <= 2:
                nc.sync.dma_start(out_ext.ap()[:, c0:c0 + w], h2s[:, :w])
            if CUT <= 1:
                continue
            dv = sb2.tile([D_OUT, CCH], f32, tag="pdv")
            nc.sync.dma_start(dv[:, :w], dinv_ext.ap()[:, c0:c0 + w])
            ah = sb2.tile([D_OUT, CCH], f32, tag="pah")
            nc.scalar.mul(ah[:, :w], h2s[:, :w], ALPHA)
            nc.sync.dma_start(ah_d[:, c0:c0 + w], ah[:, :w])
            htt = sb2.tile([D_OUT, CCH], f32, tag="phtt")
            nc.vector.tensor_mul(htt[:, :w], h2s[:, :w], dv[:, :w])
            nc.sync.dma_start(ht_d[:, c0:c0 + w], htt[:, :w])
            nc.sync.dma_start(x_d[:, c0:c0 + w], h2s[:, :w])

        # ---------------- propagation ----------------
        for step in range(K_STEPS):
            ag_d = ag_ds[step]
            nc.gpsimd.collective_compute(
                "AllGather", Alu.bypass,
                replica_groups=[list(range(N_CORES))],
                ins=[ht_d[:].opt()], outs=[ag_d[:].opt()])
            for g in range(8):
                c = g % 4
                for j in range(2):
                    nc.sync.dma_start(
                        table[16 * g:16 * g + D_OUT,
                              1 + j * ROWS:1 + (j + 1) * ROWS],
                        ag_d[:][(2 * c + j) * D_OUT:(2 * c + j + 1) * D_OUT, :])
            for t in range(N_TILES):
                ec = T_e // 16
                gbuf = sb2.tile([128, T_e], f32, tag="gbuf")
                nc.gpsimd.ap_gather(
                    gbuf[:], table[:], eidx[:, t * ec:(t + 1) * ec],
                    channels=128, num_elems=CHUNK + 1, d=1, num_idxs=T_e)
                cbuf = sb2.tile([128, T_e + 1], f32, tag="cbuf")
                nc.vector.memset(cbuf[:, 0:1], 0.0)
                nc.vector.tensor_tensor_scan(
                    cbuf[:, 1:T_e + 1], gbuf[:], gbuf[:], 0.0,
                    Alu.add, Alu.bypass)
                bc = DST_Q // 16
                barr = sb2.tile([128, BCOLS], f32, tag="barr")
                nc.vector.memset(barr[:, 0:1], 0.0)
                nc.gpsimd.ap_gather(
                    barr[:, 1:BCOLS], cbuf[:], bidx[:, t * bc:(t + 1) * bc],
                    channels=128, num_elems=T_e + 1, d=1, num_idxs=DST_Q)
                nc.vector.tensor_sub(bdiff[:, t * DST_Q:(t + 1) * DST_Q],
                                     barr[:, 1:BCOLS], barr[:, 0:DST_Q])
            for hf in range(2):
                base = hf * HALF
                for ch in range((HALF + CCH - 1) // CCH):
                    c0 = ch * CCH
                    w = min(CCH, HALF - c0)
                    sp = ps.tile([D_OUT, CCH], f32, tag="sp")
                    nc.tensor.matmul(sp[:, :w],
                                     selw[:, hf * D_OUT:(hf + 1) * D_OUT],
                                     bdiff[:, c0:c0 + w],
                                     start=True, stop=True)
                    htt = sb2.tile([D_OUT, CCH], f32, tag="phtt")
                    nc.sync.dma_start(htt[:, :w], ht_d[:, base + c0:base + c0 + w])
                    dv = sb2.tile([D_OUT, CCH], f32, tag="pdv")
                    nc.sync.dma_start(dv[:, :w],
                                      dinv_ext.ap()[:, base + c0:base + c0 + w])
                    ahh = sb2.tile([D_OUT, CCH], f32, tag="pah")
                    nc.sync.dma_start(ahh[:, :w], ah_d[:, base + c0:base + c0 + w])
                    nc.vector.tensor_add(htt[:, :w], sp[:, :w], htt[:, :w])
                    nc.vector.tensor_mul(htt[:, :w], htt[:, :w], dv[:, :w])
                    nc.vector.scalar_tensor_tensor(
                        ahh[:, :w], htt[:, :w], 1.0 - ALPHA, ahh[:, :w],
                        Alu.mult, Alu.add)
                    if step < K_STEPS - 1:
                        nc.vector.tensor_mul(htt[:, :w], ahh[:, :w], dv[:, :w])
                        nc.sync.dma_start(ht_d[:, base + c0:base + c0 + w],
                                          htt[:, :w])
                    else:
                        nc.sync.dma_start(x_d[:, base + c0:base + c0 + w],
                                          ahh[:, :w])

        # ---------------- log_softmax ----------------
        for ch in range(NRB if CUT >= 3 else 0):
            c0 = ch * CCH
            w = min(CCH, ROWS - c0)
            xt = sb2.tile([D_OUT, CCH], f32, tag="lsx")
            nc.sync.dma_start(xt[:, :w], x_d[:, c0:c0 + w])
            ex = sb2.tile([D_OUT, CCH], f32, tag="lse")
            nc.scalar.activation(ex[:, :w], xt[:, :w], Act.Exp)
            sm = ps1.tile([1, CCH], f32, tag="lssum")
            nc.tensor.matmul(sm[:, :w], ones71[:], ex[:, :w], start=True, stop=True)
            ls = sb2.tile([1, CCH], f32, tag="lsl")
            nc.scalar.activation(ls[:, :w], sm[:, :w], Act.Ln)
            lsb = ps1.tile([D_OUT, CCH], f32, tag="lsb")
            nc.tensor.matmul(lsb[:, :w], ones17[:], ls[:, :w], start=True, stop=True)
            oo = sb2.tile([D_OUT, CCH], f32, tag="lse")
            nc.vector.tensor_sub(oo[:, :w], xt[:, :w], lsb[:, :w])
            nc.sync.dma_start(out_ext.ap()[:, c0:c0 + w], oo[:, :w])

    nc.finalize()
    return nc


def kernel(x, edge_index, W1, b1, W2, b2):
    import ml_dtypes
    from concourse.bass_utils import run_bass_kernel_spmd

    x = np.asarray(x, np.float32)
    edge_index = np.asarray(edge_index)
    W1 = np.asarray(W1, np.float32)
    b1 = np.asarray(b1, np.float32)
    W2 = np.asarray(W2, np.float32)
    b2 = np.asarray(b2, np.float32)

    src = edge_index[0].astype(np.int64)
    dst = edge_index[1].astype(np.int64)

    deg = np.bincount(dst, minlength=N_NODES).astype(np.float64) + 1.0
    dinv = (1.0 / np.sqrt(deg)).astype(np.float32)

    order_all = np.argsort(dst, kind="stable")
    src_s, dst_s = src[order_all], dst[order_all]
    core_of = dst_s // ROWS
    T_e = 0
    per_core = []
    for cidx in range(N_CORES):
        m = core_of == cidx
        s, d = src_s[m], dst_s[m] - cidx * ROWS
        per_core.append((s, d))
        grp = (s // CHUNK) + 4 * (d // HALF)
        tl = (d % HALF) // DST_Q
        cnt = np.bincount(grp * N_TILES + tl, minlength=8 * N_TILES)
        T_e = max(T_e, int(cnt.max()))
    T_e = ((T_e + 15) // 16) * 16

    nc = _build_bass(T_e)

    xT = np.zeros((D_IN_PAD, N_NODES), ml_dtypes.bfloat16)
    xT[:D_IN] = x.T.astype(ml_dtypes.bfloat16)
    W1p = np.zeros((D_IN_PAD, D_HID), np.float32)
    W1p[:D_IN] = W1
    W1b = np.ascontiguousarray(
        W1p.astype(ml_dtypes.bfloat16).reshape(D_IN_PAD // 128, 128, D_HID)
        .transpose(1, 0, 2).reshape(128, (D_IN_PAD // 128) * D_HID))
    W2b = W2.astype(ml_dtypes.bfloat16)
    sel = np.zeros((128, 2 * D_OUT), np.float32)
    for hf in range(2):
        for g in range(4 * hf, 4 * hf + 4):
            for f in range(D_OUT):
                sel[16 * g + f, hf * D_OUT + f] = 1.0

    in_maps = []
    for cidx in range(N_CORES):
        s, d = per_core[cidx]
        eidx, bidx = _build_core_plan(s, d, T_e)
        sl = slice(cidx * ROWS, (cidx + 1) * ROWS)
        dinv7 = np.broadcast_to(dinv[sl], (D_OUT, ROWS)).copy()
        in_maps.append({
            "xT": np.ascontiguousarray(xT[:, sl]),
            "w1": W1b, "b1": b1.reshape(D_HID, 1).astype(np.float32),
            "w2": W2b, "b2": b2.reshape(D_OUT, 1).astype(np.float32),
            "dinv7": dinv7, "eidx": eidx, "bidx": bidx, "sel": sel,
        })

    res = run_bass_kernel_spmd(nc, in_maps, core_ids=list(range(N_CORES)))
    out = np.concatenate([res.results[c]["out"].T for c in range(N_CORES)], axis=0)
    return out.astype(np.float32)
